# revision 20
# baseline (speedup 1.0000x reference)
"""MoE (top-2 of 8 experts, gelu MLP) on 8 TRN2 NeuronCores.

Strategy (expert-parallel, per the sharding hint):
  Host dispatch: router scores/top-2/softmax on host (the router is 0.05% of
    the FLOPs; "replicate router" per the hint — the host mediates the
    all-to-all anyway). Each expert's selected token columns are gathered
    into a capacity-padded bf16 batch.
  Device (expert-parallel, one launch): each core runs one expert's FFN
    out = gelu(xsel @ w1[e].T) @ w2[e].T over its gathered tokens in bf16
    (fp32 PSUM accumulation), with:
      - x resident in SBUF across all 8 FFN column blocks (no re-DMA),
      - PE warm-up matmuls covering the initial DMA wait + p-state ramp,
      - q0's w1 and the first tokens host-packed dense (728ns DMAs) and the
        first two token tiles' mm1 hoisted ahead of mm2, so real matmuls
        start ~4.3us in and never starve on the w2 load,
      - output written during the last FFN block (acc add then DMA per
        512-wide half, alternating the two HWDGE queues; the final chain
        split into 256-wide halves) to minimize the tail.
  Host combine: scatter-add with the routing weight applied during the
  combine (each token appears in exactly two experts' batches).

kernel(**inputs) -> np.ndarray  takes FULL inputs, returns FULL output.
"""

import numpy as np
import ml_dtypes

import concourse.bass as bass
import concourse.mybir as mybir
from concourse import bacc
from concourse.tile import TileContext
from concourse.bass_utils import run_bass_kernel_spmd

HIDDEN = 1024
NUM_EXPERTS = 8
TOP_K = 2
FFN = 4096
BATCH, SEQ = 4, 2048
T = BATCH * SEQ          # 8192 tokens
NCORES = 8
P = 128
DK = HIDDEN // P         # 8 contraction tiles over hidden
FQ = 8                   # FFN column blocks
FQ_SIZE = FFN // FQ      # 512
FM = FQ_SIZE // P        # 4 f-subtiles per block
DN = HIDDEN // 512       # 2 output column halves
MAXC = 2560              # SBUF limit for resident x + fp32 accumulator
NWARM = 10               # PE warm-up matmuls (512 rows each)

f32 = mybir.dt.float32
bf16 = mybir.dt.bfloat16


def _to_bf16(a: np.ndarray) -> np.ndarray:
    """fp32 -> bf16 with round-to-nearest-even (fast uint path)."""
    u = np.ascontiguousarray(a, dtype=np.float32).view(np.uint32)
    r = ((u + np.uint32(0x7FFF) + ((u >> np.uint32(16)) & np.uint32(1)))
         >> np.uint32(16)).astype(np.uint16)
    return r.view(ml_dtypes.bfloat16)


def _build_expert_ffn(C: int, nwarm: int = NWARM):
    """Per core: one expert's FFN over C gathered tokens (bf16 matmuls).

    inputs: xt  [HIDDEN, C] bf16    gathered+transposed tokens
            w1h [P, FM*DK*P] bf16   w1t[:, 0:512] host-packed [ki][(fm ko f)]
            x0p [P, DK*P] bf16      xt[:, 0:128] host-packed [ki][(ko t)]
            w1t [HIDDEN, FFN] bf16  expert w1 transposed
            w2t [FFN, HIDDEN] bf16  expert w2 transposed
    output: o   [C, HIDDEN] f32     gelu(x @ w1.T) @ w2.T   (ungated)
    """
    assert C % P == 0 and C >= 1024
    act = mybir.ActivationFunctionType.Gelu
    nc = bacc.Bacc(None)
    xt_d = nc.declare_dram_parameter("xt", [HIDDEN, C], bf16, isOutput=False)
    w1h_d = nc.declare_dram_parameter("w1h", [P, FM * DK * P], bf16, isOutput=False)
    x0p_d = nc.declare_dram_parameter("x0p", [P, DK * P], bf16, isOutput=False)
    w1t_d = nc.declare_dram_parameter("w1t", [HIDDEN, FFN], bf16, isOutput=False)
    w2t_d = nc.declare_dram_parameter("w2t", [FFN, HIDDEN], bf16, isOutput=False)
    o_d = nc.declare_dram_parameter("o", [C, HIDDEN], f32, isOutput=True)

    MT = C // P
    # token tiles: q0 starts fine-grained (128 host-packed + two 256s) so the
    # first matmuls only wait for small DMAs; later q use full 512 tiles.
    t_sizes0 = [128, 256, 256] + [512] * ((C - 640) // 512)
    if (C - 640) % 512:
        t_sizes0.append((C - 640) % 512)
    t_sizesN = [512] * (C // 512)
    if C % 512:
        t_sizesN.append(C % 512)

    with TileContext(nc) as tc:
        with tc.tile_pool(name="big", bufs=1) as bigp, \
             tc.tile_pool(name="w1p", bufs=3) as w1p, \
             tc.tile_pool(name="w2p", bufs=3) as w2p, \
             tc.tile_pool(name="h", bufs=3) as hp, \
             tc.tile_pool(name="ps1", bufs=4, space="PSUM") as ps1p, \
             tc.tile_pool(name="ps2", bufs=4, space="PSUM") as ps2p:
            x_t = bigp.tile([P, DK, C], bf16)
            acc = bigp.tile([P, MT, HIDDEN], f32)
            warm = bigp.tile([P, 512], bf16)
            w1h = bigp.tile([P, FM, DK, P], bf16)
            x0p = bigp.tile([P, DK, P], bf16)

            x_r = xt_d[:].rearrange("(ko ki) t -> ki ko t", ki=P)
            w1h_r = w1h_d[:].rearrange("p (fm ko f) -> p fm ko f", fm=FM, ko=DK)

            # -- critical-path DMAs: q0's w1 host-packed as 4 dense 728ns
            #    chunks + the first 128 tokens packed; then 512B-run chunks.
            nc.sync.dma_start(out=w1h[:, 0], in_=w1h_r[:, 0])
            nc.sync.dma_start(out=x0p[:], in_=x0p_d[:].rearrange("p (ko t) -> p ko t", ko=DK))
            for fm in range(1, FM):
                nc.sync.dma_start(out=w1h[:, fm], in_=w1h_r[:, fm])
            nc.sync.dma_start(out=x_t[:, :, 128:384], in_=x_r[:, :, 128:384])
            w2q0 = w2p.tile([P, FM, HIDDEN], bf16, tag="w2q")
            w2r0 = w2t_d[0:FQ_SIZE, :].rearrange("(fo fi) d -> fi fo d", fi=P)
            for fk in range(FM):
                nc.sync.dma_start(out=w2q0[:, fk], in_=w2r0[:, fk])
            for off in range(384, C, 512):
                sz = min(512, C - off)
                nc.sync.dma_start(out=x_t[:, :, off:off + sz],
                                  in_=x_r[:, :, off:off + sz])
            # tokens 0:128 land in x_t too (q>=1 reads them from x_t)
            nc.sync.dma_start(out=x_t[:, :, 0:128], in_=x_r[:, :, 0:128])

            # -- PE warm-up: matmuls on a zeroed tile keep the PE busy (and
            #    ramp its p-state) while the first loads land.
            nc.vector.memset(warm[:], 0.0)
            wps = ps1p.tile([P, 512], f32, tag="ph", name="wps")
            for i in range(nwarm):
                nc.tensor.matmul(wps[:], warm[:, 0:P], warm[:],
                                 start=(i == 0), stop=(i == nwarm - 1))

            o_r = o_d[:].rearrange("(mo p) d -> p mo d", p=P)

            def do_mm1(q, w1q, t_off, t_size):
                h_t = hp.tile([P, FM, 512], bf16, tag="h", name="h_t")[:, :, :t_size]
                for fm in range(FM):
                    ph = ps1p.tile([P, 512], f32, tag="ph", name="ph")[:, :t_size]
                    for k in range(DK):
                        w1ap = (w1h[:, fm, k] if q == 0
                                else w1q[:, k, fm * P:(fm + 1) * P])
                        xap = (x0p[:, k] if q == 0 and t_off == 0
                               else x_t[:, k, t_off:t_off + t_size])
                        nc.tensor.matmul(ph[:], w1ap, xap,
                                         start=(k == 0), stop=(k == DK - 1))
                    nc.scalar.activation(h_t[:, fm], ph[:], act)
                return h_t

            def do_mm2(q, w2q, t_off, t_size, h_t):
                for tm in range(t_size // P):
                    mt = t_off // P + tm
                    last_mt = (q == FQ - 1) and (mt == MT - 1)
                    for dn in range(DN):
                        po = ps2p.tile([P, 512], f32, tag="po")
                        for fk in range(FM):
                            nc.tensor.matmul(po[:],
                                             h_t[:, fk, tm * P:(tm + 1) * P],
                                             w2q[:, fk, dn * 512:(dn + 1) * 512],
                                             start=(fk == 0), stop=(fk == FM - 1))
                        dcol = slice(dn * 512, (dn + 1) * 512)
                        if q == 0:
                            nc.vector.tensor_copy(acc[:, mt, dcol], po[:])
                        elif last_mt and dn == DN - 1:
                            # final chain: 256-wide halves, stores split
                            # across both HWDGE queues
                            for hh, eng, dq in ((0, nc.vector, nc.scalar),
                                                (1, nc.vector, nc.sync)):
                                cl = slice(dn * 512 + hh * 256,
                                           dn * 512 + (hh + 1) * 256)
                                eng.tensor_add(acc[:, mt, cl],
                                               acc[:, mt, cl],
                                               po[:, hh * 256:(hh + 1) * 256])
                                dq.dma_start(out=o_r[:, mt, cl],
                                             in_=acc[:, mt, cl])
                        else:
                            nc.vector.tensor_add(acc[:, mt, dcol],
                                                 acc[:, mt, dcol], po[:])
                            if q == FQ - 1:
                                dq = nc.scalar if dn == 0 else nc.sync
                                dq.dma_start(out=o_r[:, mt, dcol],
                                             in_=acc[:, mt, dcol])

            for q in range(FQ):
                if q == 0:
                    w1q, w2q = None, w2q0
                else:
                    w1q = w1p.tile([P, DK, FQ_SIZE], bf16, tag="w1q")
                    w1r = w1t_d[:, q * FQ_SIZE:(q + 1) * FQ_SIZE].rearrange(
                        "(ko ki) f -> ki ko f", ki=P)
                    nc.sync.dma_start(out=w1q[:], in_=w1r)
                    w2q = w2p.tile([P, FM, HIDDEN], bf16, tag="w2q")
                    w2r = w2t_d[q * FQ_SIZE:(q + 1) * FQ_SIZE, :].rearrange(
                        "(fo fi) d -> fi fo d", fi=P)
                    for fk in range(FM):
                        nc.sync.dma_start(out=w2q[:, fk], in_=w2r[:, fk])

                t_sizes = t_sizes0 if q == 0 else t_sizesN
                t_offs = np.cumsum([0] + t_sizes)[:-1].tolist()
                if q == 0:
                    # interleave the first two (small) tiles: both mm1 groups
                    # run before mm2 needs w2, hiding the w2(q0) load.
                    h0 = do_mm1(q, w1q, t_offs[0], t_sizes[0])
                    h1 = do_mm1(q, w1q, t_offs[1], t_sizes[1])
                    do_mm2(q, w2q, t_offs[0], t_sizes[0], h0)
                    do_mm2(q, w2q, t_offs[1], t_sizes[1], h1)
                    rest = list(zip(t_offs, t_sizes))[2:]
                else:
                    rest = list(zip(t_offs, t_sizes))
                for t_off, t_size in rest:
                    h_t = do_mm1(q, w1q, t_off, t_size)
                    do_mm2(q, w2q, t_off, t_size, h_t)
    nc.compile()
    return nc


_B_CACHE = {}
LAST_HW_NS = None


def _run_spmd(nc, in_maps, retries=2):
    """run_bass_kernel_spmd with retry: device crashes on this axon path are
    occasionally transient (NRT_EXEC_UNIT_UNRECOVERABLE recovers on a fresh
    attempt)."""
    last = None
    for attempt in range(retries + 1):
        try:
            return run_bass_kernel_spmd(nc, in_maps, list(range(NCORES)))
        except Exception as e:  # noqa: BLE001
            last = e
            import time as _time
            _time.sleep(2.0 * (attempt + 1))
    raise last


def _expert_ffn_nc(C):
    if C not in _B_CACHE:
        _B_CACHE[C] = _build_expert_ffn(C)
    return _B_CACHE[C]


def kernel(x, router_w, expert_w1, expert_w2):
    xf = np.ascontiguousarray(x.reshape(T, HIDDEN), dtype=np.float32)

    # ---- host dispatch: router (replicated per the hint) + top-2 softmax ----
    scores = xf @ np.ascontiguousarray(router_w.astype(np.float32)).T  # [T, E]
    top1 = np.argmax(scores, axis=1)
    rng = np.arange(T)
    s1 = scores[rng, top1]
    scores2 = scores.copy()
    scores2[rng, top1] = -np.inf
    top2 = np.argmax(scores2, axis=1)
    s2 = scores2[rng, top2]
    g1 = 1.0 / (1.0 + np.exp(-(s1 - s2)))     # softmax over the two slots
    w_all = np.zeros((T, NUM_EXPERTS), dtype=np.float32)
    w_all[rng, top1] = g1
    w_all[rng, top2] = 1.0 - g1

    idx = [np.nonzero((top1 == e) | (top2 == e))[0] for e in range(NUM_EXPERTS)]
    cmax = max(len(i) for i in idx)
    C = min(((cmax + P - 1) // P) * P, MAXC)
    n_chunks = (cmax + C - 1) // C

    xT16 = _to_bf16(xf).T                      # [D, T] bf16 view (strided ok)
    w1t16 = [np.ascontiguousarray(_to_bf16(expert_w1[e]).T) for e in range(NUM_EXPERTS)]
    w2t16 = [np.ascontiguousarray(_to_bf16(expert_w2[e]).T) for e in range(NUM_EXPERTS)]
    # w1h: w1t[:, 0:512] packed [ki][(fm ko f)] (dense rows for a fast start)
    w1h16 = [np.ascontiguousarray(
        w[:, :FQ_SIZE].reshape(DK, P, FM, P).transpose(1, 2, 0, 3)
        .reshape(P, FM * DK * P)) for w in w1t16]
    nc_b = _expert_ffn_nc(C)

    out = np.zeros((T, HIDDEN), dtype=np.float32)
    for r in range(n_chunks):
        in_b = []
        for e in range(NUM_EXPERTS):
            ids = idx[e][r * C:(r + 1) * C]
            n = len(ids)
            xsel = np.zeros((HIDDEN, C), dtype=ml_dtypes.bfloat16)
            xsel[:, :n] = xT16[:, ids]
            x0p = np.ascontiguousarray(
                xsel[:, :P].reshape(DK, P, P).transpose(1, 0, 2).reshape(P, DK * P))
            in_b.append({"xt": xsel, "w1h": w1h16[e], "x0p": x0p,
                         "w1t": w1t16[e], "w2t": w2t16[e]})
        # ---- expert-parallel FFN on device (single launch per chunk) ----
        res_b = _run_spmd(nc_b, in_b)
        # ---- host combine: gated scatter-add (ids unique per expert) ----
        for e in range(NUM_EXPERTS):
            ids = idx[e][r * C:(r + 1) * C]
            n = len(ids)
            out[ids] += res_b.results[e]["o"][:n] * w_all[ids, e][:, None]

    # cost-model exec-time estimate (NTFF profiling unavailable on this path)
    global LAST_HW_NS
    try:
        if ("t", C) not in _B_CACHE:
            from concourse.timeline_sim import TimelineSim
            _B_CACHE[("t", C)] = TimelineSim(nc_b).simulate() * n_chunks
        LAST_HW_NS = int(_B_CACHE[("t", C)])
    except Exception:  # noqa: BLE001
        pass
    return out.reshape(BATCH, SEQ, HIDDEN)


# revision 21
# speedup vs baseline: 1.0001x; 1.0001x over previous
"""MoE (top-2 of 8 experts, gelu MLP) on 8 TRN2 NeuronCores.

Strategy (expert-parallel, per the sharding hint):
  Host dispatch: router scores/top-2/softmax on host (the router is 0.05% of
    the FLOPs; "replicate router" per the hint — the host mediates the
    all-to-all anyway). Each expert's selected token columns are gathered
    into a capacity-padded bf16 batch.
  Device (expert-parallel, one launch): each core runs one expert's FFN
    out = gelu(xsel @ w1[e].T) @ w2[e].T over its gathered tokens in bf16
    (fp32 PSUM accumulation), with:
      - x resident in SBUF across all 8 FFN column blocks (no re-DMA),
      - PE warm-up matmuls covering the initial DMA wait + p-state ramp,
      - q0's w1 and the first tokens host-packed dense (728ns DMAs) and the
        first two token tiles' mm1 hoisted ahead of mm2, so real matmuls
        start ~4.3us in and never starve on the w2 load,
      - output written during the last FFN block (acc add then DMA per
        512-wide half, alternating the two HWDGE queues; the final chain
        split into 256-wide halves) to minimize the tail.
  Host combine: scatter-add with the routing weight applied during the
  combine (each token appears in exactly two experts' batches).

kernel(**inputs) -> np.ndarray  takes FULL inputs, returns FULL output.
"""

import numpy as np
import ml_dtypes

import concourse.bass as bass
import concourse.mybir as mybir
from concourse import bacc
from concourse.tile import TileContext
from concourse.bass_utils import run_bass_kernel_spmd

HIDDEN = 1024
NUM_EXPERTS = 8
TOP_K = 2
FFN = 4096
BATCH, SEQ = 4, 2048
T = BATCH * SEQ          # 8192 tokens
NCORES = 8
P = 128
DK = HIDDEN // P         # 8 contraction tiles over hidden
FQ = 8                   # FFN column blocks
FQ_SIZE = FFN // FQ      # 512
FM = FQ_SIZE // P        # 4 f-subtiles per block
DN = HIDDEN // 512       # 2 output column halves
MAXC = 2560              # SBUF limit for resident x + fp32 accumulator
NWARM = 10               # PE warm-up matmuls (512 rows each)

f32 = mybir.dt.float32
bf16 = mybir.dt.bfloat16


def _to_bf16(a: np.ndarray) -> np.ndarray:
    """fp32 -> bf16 with round-to-nearest-even (fast uint path)."""
    u = np.ascontiguousarray(a, dtype=np.float32).view(np.uint32)
    r = ((u + np.uint32(0x7FFF) + ((u >> np.uint32(16)) & np.uint32(1)))
         >> np.uint32(16)).astype(np.uint16)
    return r.view(ml_dtypes.bfloat16)


def _build_expert_ffn(C: int, nwarm: int = NWARM):
    """Per core: one expert's FFN over C gathered tokens (bf16 matmuls).

    inputs: xt  [HIDDEN, C] bf16    gathered+transposed tokens
            w1h [P, FM*DK*P] bf16   w1t[:, 0:512] host-packed [ki][(fm ko f)]
            x0p [P, DK*P] bf16      xt[:, 0:128] host-packed [ki][(ko t)]
            w1t [HIDDEN, FFN] bf16  expert w1 transposed
            w2t [FFN, HIDDEN] bf16  expert w2 transposed
    output: o   [C, HIDDEN] f32     gelu(x @ w1.T) @ w2.T   (ungated)
    """
    assert C % P == 0 and C >= 1024
    act = mybir.ActivationFunctionType.Gelu
    nc = bacc.Bacc(None)
    xt_d = nc.declare_dram_parameter("xt", [HIDDEN, C], bf16, isOutput=False)
    w1h_d = nc.declare_dram_parameter("w1h", [P, FM * DK * P], bf16, isOutput=False)
    x0p_d = nc.declare_dram_parameter("x0p", [P, DK * P], bf16, isOutput=False)
    w1t_d = nc.declare_dram_parameter("w1t", [HIDDEN, FFN], bf16, isOutput=False)
    w2t_d = nc.declare_dram_parameter("w2t", [FFN, HIDDEN], bf16, isOutput=False)
    o_d = nc.declare_dram_parameter("o", [C, HIDDEN], f32, isOutput=True)

    MT = C // P
    # token tiles: q0 starts fine-grained (128 host-packed + two 256s) so the
    # first matmuls only wait for small DMAs; later q use full 512 tiles.
    t_sizes0 = [128, 256, 256] + [512] * ((C - 640) // 512)
    if (C - 640) % 512:
        t_sizes0.append((C - 640) % 512)
    t_sizesN = [512] * (C // 512)
    if C % 512:
        t_sizesN.append(C % 512)

    with TileContext(nc) as tc:
        with tc.tile_pool(name="big", bufs=1) as bigp, \
             tc.tile_pool(name="w1p", bufs=3) as w1p, \
             tc.tile_pool(name="w2p", bufs=3) as w2p, \
             tc.tile_pool(name="h", bufs=3) as hp, \
             tc.tile_pool(name="ps1", bufs=4, space="PSUM") as ps1p, \
             tc.tile_pool(name="ps2", bufs=4, space="PSUM") as ps2p:
            x_t = bigp.tile([P, DK, C], bf16)
            acc = bigp.tile([P, MT, HIDDEN], f32)
            warm = bigp.tile([P, 512], bf16)
            w1h = bigp.tile([P, FM, DK, P], bf16)
            x0p = bigp.tile([P, DK, P], bf16)

            x_r = xt_d[:].rearrange("(ko ki) t -> ki ko t", ki=P)
            w1h_r = w1h_d[:].rearrange("p (fm ko f) -> p fm ko f", fm=FM, ko=DK)

            # -- critical-path DMAs: q0's w1 host-packed as 4 dense 728ns
            #    chunks + the first 128 tokens packed; then 512B-run chunks.
            nc.sync.dma_start(out=w1h[:, 0], in_=w1h_r[:, 0])
            nc.sync.dma_start(out=x0p[:], in_=x0p_d[:].rearrange("p (ko t) -> p ko t", ko=DK))
            for fm in range(1, FM):
                nc.sync.dma_start(out=w1h[:, fm], in_=w1h_r[:, fm])
            nc.sync.dma_start(out=x_t[:, :, 128:384], in_=x_r[:, :, 128:384])
            w2q0 = w2p.tile([P, FM, HIDDEN], bf16, tag="w2q")
            w2r0 = w2t_d[0:FQ_SIZE, :].rearrange("(fo fi) d -> fi fo d", fi=P)
            for fk in range(FM):
                nc.sync.dma_start(out=w2q0[:, fk], in_=w2r0[:, fk])
            for off in range(384, C, 512):
                sz = min(512, C - off)
                nc.sync.dma_start(out=x_t[:, :, off:off + sz],
                                  in_=x_r[:, :, off:off + sz])
            # tokens 0:128 land in x_t too (q>=1 reads them from x_t)
            nc.sync.dma_start(out=x_t[:, :, 0:128], in_=x_r[:, :, 0:128])

            # -- PE warm-up: matmuls on a zeroed tile keep the PE busy (and
            #    ramp its p-state) while the first loads land.
            nc.vector.memset(warm[:], 0.0)
            wps = ps1p.tile([P, 512], f32, tag="ph", name="wps")
            for i in range(nwarm):
                nc.tensor.matmul(wps[:], warm[:, 0:P], warm[:],
                                 start=(i == 0), stop=(i == nwarm - 1))

            o_r = o_d[:].rearrange("(mo p) d -> p mo d", p=P)

            def do_mm1(q, w1q, t_off, t_size):
                h_t = hp.tile([P, FM, 512], bf16, tag="h", name="h_t")[:, :, :t_size]
                for fm in range(FM):
                    ph = ps1p.tile([P, 512], f32, tag="ph", name="ph")[:, :t_size]
                    for k in range(DK):
                        w1ap = (w1h[:, fm, k] if q == 0
                                else w1q[:, k, fm * P:(fm + 1) * P])
                        xap = (x0p[:, k] if q == 0 and t_off == 0
                               else x_t[:, k, t_off:t_off + t_size])
                        nc.tensor.matmul(ph[:], w1ap, xap,
                                         start=(k == 0), stop=(k == DK - 1))
                    nc.scalar.activation(h_t[:, fm], ph[:], act)
                return h_t

            def do_mm2(q, w2q, t_off, t_size, h_t):
                for tm in range(t_size // P):
                    mt = t_off // P + tm
                    last_mt = (q == FQ - 1) and (mt == MT - 1)
                    for dn in range(DN):
                        po = ps2p.tile([P, 512], f32, tag="po")
                        for fk in range(FM):
                            nc.tensor.matmul(po[:],
                                             h_t[:, fk, tm * P:(tm + 1) * P],
                                             w2q[:, fk, dn * 512:(dn + 1) * 512],
                                             start=(fk == 0), stop=(fk == FM - 1))
                        dcol = slice(dn * 512, (dn + 1) * 512)
                        if q == 0:
                            nc.vector.tensor_copy(acc[:, mt, dcol], po[:])
                        elif last_mt and dn == DN - 1:
                            nc.vector.tensor_add(acc[:, mt, dcol],
                                                 acc[:, mt, dcol], po[:])
                            nc.sync.dma_start(out=o_r[:, mt, dcol],
                                              in_=acc[:, mt, dcol])
                        else:
                            nc.vector.tensor_add(acc[:, mt, dcol],
                                                 acc[:, mt, dcol], po[:])
                            if q == FQ - 1:
                                dq = nc.scalar if dn == 0 else nc.sync
                                dq.dma_start(out=o_r[:, mt, dcol],
                                             in_=acc[:, mt, dcol])

            for q in range(FQ):
                if q == 0:
                    w1q, w2q = None, w2q0
                else:
                    w1q = w1p.tile([P, DK, FQ_SIZE], bf16, tag="w1q")
                    w1r = w1t_d[:, q * FQ_SIZE:(q + 1) * FQ_SIZE].rearrange(
                        "(ko ki) f -> ki ko f", ki=P)
                    nc.sync.dma_start(out=w1q[:], in_=w1r)
                    w2q = w2p.tile([P, FM, HIDDEN], bf16, tag="w2q")
                    w2r = w2t_d[q * FQ_SIZE:(q + 1) * FQ_SIZE, :].rearrange(
                        "(fo fi) d -> fi fo d", fi=P)
                    for fk in range(FM):
                        nc.sync.dma_start(out=w2q[:, fk], in_=w2r[:, fk])

                t_sizes = t_sizes0 if q == 0 else t_sizesN
                t_offs = np.cumsum([0] + t_sizes)[:-1].tolist()
                if q == 0:
                    # interleave the first two (small) tiles: both mm1 groups
                    # run before mm2 needs w2, hiding the w2(q0) load.
                    h0 = do_mm1(q, w1q, t_offs[0], t_sizes[0])
                    h1 = do_mm1(q, w1q, t_offs[1], t_sizes[1])
                    do_mm2(q, w2q, t_offs[0], t_sizes[0], h0)
                    do_mm2(q, w2q, t_offs[1], t_sizes[1], h1)
                    rest = list(zip(t_offs, t_sizes))[2:]
                else:
                    rest = list(zip(t_offs, t_sizes))
                for t_off, t_size in rest:
                    h_t = do_mm1(q, w1q, t_off, t_size)
                    do_mm2(q, w2q, t_off, t_size, h_t)
    nc.compile()
    return nc


_B_CACHE = {}
LAST_HW_NS = None


def _run_spmd(nc, in_maps, retries=2):
    """run_bass_kernel_spmd with retry: device crashes on this axon path are
    occasionally transient (NRT_EXEC_UNIT_UNRECOVERABLE recovers on a fresh
    attempt)."""
    last = None
    for attempt in range(retries + 1):
        try:
            return run_bass_kernel_spmd(nc, in_maps, list(range(NCORES)))
        except Exception as e:  # noqa: BLE001
            last = e
            import time as _time
            _time.sleep(2.0 * (attempt + 1))
    raise last


def _expert_ffn_nc(C):
    if C not in _B_CACHE:
        _B_CACHE[C] = _build_expert_ffn(C)
    return _B_CACHE[C]


def kernel(x, router_w, expert_w1, expert_w2):
    xf = np.ascontiguousarray(x.reshape(T, HIDDEN), dtype=np.float32)

    # ---- host dispatch: router (replicated per the hint) + top-2 softmax ----
    scores = xf @ np.ascontiguousarray(router_w.astype(np.float32)).T  # [T, E]
    top1 = np.argmax(scores, axis=1)
    rng = np.arange(T)
    s1 = scores[rng, top1]
    scores2 = scores.copy()
    scores2[rng, top1] = -np.inf
    top2 = np.argmax(scores2, axis=1)
    s2 = scores2[rng, top2]
    g1 = 1.0 / (1.0 + np.exp(-(s1 - s2)))     # softmax over the two slots
    w_all = np.zeros((T, NUM_EXPERTS), dtype=np.float32)
    w_all[rng, top1] = g1
    w_all[rng, top2] = 1.0 - g1

    idx = [np.nonzero((top1 == e) | (top2 == e))[0] for e in range(NUM_EXPERTS)]
    cmax = max(len(i) for i in idx)
    C = min(((cmax + P - 1) // P) * P, MAXC)
    n_chunks = (cmax + C - 1) // C

    xT16 = _to_bf16(xf).T                      # [D, T] bf16 view (strided ok)
    w1t16 = [np.ascontiguousarray(_to_bf16(expert_w1[e]).T) for e in range(NUM_EXPERTS)]
    w2t16 = [np.ascontiguousarray(_to_bf16(expert_w2[e]).T) for e in range(NUM_EXPERTS)]
    # w1h: w1t[:, 0:512] packed [ki][(fm ko f)] (dense rows for a fast start)
    w1h16 = [np.ascontiguousarray(
        w[:, :FQ_SIZE].reshape(DK, P, FM, P).transpose(1, 2, 0, 3)
        .reshape(P, FM * DK * P)) for w in w1t16]
    nc_b = _expert_ffn_nc(C)

    out = np.zeros((T, HIDDEN), dtype=np.float32)
    for r in range(n_chunks):
        in_b = []
        for e in range(NUM_EXPERTS):
            ids = idx[e][r * C:(r + 1) * C]
            n = len(ids)
            xsel = np.zeros((HIDDEN, C), dtype=ml_dtypes.bfloat16)
            xsel[:, :n] = xT16[:, ids]
            x0p = np.ascontiguousarray(
                xsel[:, :P].reshape(DK, P, P).transpose(1, 0, 2).reshape(P, DK * P))
            in_b.append({"xt": xsel, "w1h": w1h16[e], "x0p": x0p,
                         "w1t": w1t16[e], "w2t": w2t16[e]})
        # ---- expert-parallel FFN on device (single launch per chunk) ----
        res_b = _run_spmd(nc_b, in_b)
        # ---- host combine: gated scatter-add (ids unique per expert) ----
        for e in range(NUM_EXPERTS):
            ids = idx[e][r * C:(r + 1) * C]
            n = len(ids)
            out[ids] += res_b.results[e]["o"][:n] * w_all[ids, e][:, None]

    # cost-model exec-time estimate (NTFF profiling unavailable on this path)
    global LAST_HW_NS
    try:
        if ("t", C) not in _B_CACHE:
            from concourse.timeline_sim import TimelineSim
            _B_CACHE[("t", C)] = TimelineSim(nc_b).simulate() * n_chunks
        LAST_HW_NS = int(_B_CACHE[("t", C)])
    except Exception:  # noqa: BLE001
        pass
    return out.reshape(BATCH, SEQ, HIDDEN)


# revision 24
# speedup vs baseline: 1.0039x; 1.0038x over previous
"""MoE (top-2 of 8 experts, gelu MLP) on 8 TRN2 NeuronCores.

Strategy (expert-parallel, per the sharding hint):
  Host dispatch: router scores/top-2/softmax on host (the router is 0.05% of
    the FLOPs; "replicate router" per the hint — the host mediates the
    all-to-all anyway). Each expert's selected token columns are gathered
    into a capacity-padded bf16 batch.
  Device (expert-parallel, one launch): each core runs one expert's FFN
    out = gelu(xsel @ w1[e].T) @ w2[e].T over its gathered tokens in bf16
    (fp32 PSUM accumulation), with:
      - x resident in SBUF across all 8 FFN column blocks (no re-DMA),
      - PE warm-up matmuls covering the initial DMA wait + p-state ramp,
      - q0's w1 and the first tokens host-packed dense (728ns DMAs) and the
        first two token tiles' mm1 hoisted ahead of mm2, so real matmuls
        start ~4.3us in and never starve on the w2 load,
      - output written during the last FFN block (acc add then DMA per
        512-wide half, alternating the two HWDGE queues; the final chain
        split into 256-wide halves) to minimize the tail.
  Host combine: scatter-add with the routing weight applied during the
  combine (each token appears in exactly two experts' batches).

kernel(**inputs) -> np.ndarray  takes FULL inputs, returns FULL output.
"""

import numpy as np
import ml_dtypes

import concourse.bass as bass
import concourse.mybir as mybir
from concourse import bacc
from concourse.tile import TileContext
from concourse.bass_utils import run_bass_kernel_spmd

HIDDEN = 1024
NUM_EXPERTS = 8
TOP_K = 2
FFN = 4096
BATCH, SEQ = 4, 2048
T = BATCH * SEQ          # 8192 tokens
NCORES = 8
P = 128
DK = HIDDEN // P         # 8 contraction tiles over hidden
FQ = 8                   # FFN column blocks
FQ_SIZE = FFN // FQ      # 512
FM = FQ_SIZE // P        # 4 f-subtiles per block
DN = HIDDEN // 512       # 2 output column halves
MAXC = 2560              # SBUF limit for resident x + fp32 accumulator
NWARM = 10               # PE warm-up matmuls (512 rows each)

f32 = mybir.dt.float32
bf16 = mybir.dt.bfloat16


def _to_bf16(a: np.ndarray) -> np.ndarray:
    """fp32 -> bf16 with round-to-nearest-even (fast uint path)."""
    u = np.ascontiguousarray(a, dtype=np.float32).view(np.uint32)
    r = ((u + np.uint32(0x7FFF) + ((u >> np.uint32(16)) & np.uint32(1)))
         >> np.uint32(16)).astype(np.uint16)
    return r.view(ml_dtypes.bfloat16)


def _build_expert_ffn(C: int, nwarm: int = NWARM):
    """Per core: one expert's FFN over C gathered tokens (bf16 matmuls).

    inputs: xt  [HIDDEN, C] bf16    gathered+transposed tokens
            w1h [P, FM*DK*P] bf16   w1t[:, 0:512] host-packed [ki][(fm ko f)]
            x0p [P, DK*P] bf16      xt[:, 0:128] host-packed [ki][(ko t)]
            w1t [HIDDEN, FFN] bf16  expert w1 transposed
            w2t [FFN, HIDDEN] bf16  expert w2 transposed
    output: o   [C, HIDDEN] bf16    gelu(x @ w1.T) @ w2.T   (ungated)
    """
    assert C % P == 0 and C >= 1024
    act = mybir.ActivationFunctionType.Gelu
    nc = bacc.Bacc(None)
    xt_d = nc.declare_dram_parameter("xt", [HIDDEN, C], bf16, isOutput=False)
    w1h_d = nc.declare_dram_parameter("w1h", [P, FM * DK * P], bf16, isOutput=False)
    x0p_d = nc.declare_dram_parameter("x0p", [P, DK * P], bf16, isOutput=False)
    w1t_d = nc.declare_dram_parameter("w1t", [HIDDEN, FFN], bf16, isOutput=False)
    w2t_d = nc.declare_dram_parameter("w2t", [FFN, HIDDEN], bf16, isOutput=False)
    o_d = nc.declare_dram_parameter("o", [C, HIDDEN], bf16, isOutput=True)

    MT = C // P
    # token tiles: q0 starts fine-grained (128 host-packed + two 256s) so the
    # first matmuls only wait for small DMAs; later q use full 512 tiles.
    t_sizes0 = [128, 256, 256] + [512] * ((C - 640) // 512)
    if (C - 640) % 512:
        t_sizes0.append((C - 640) % 512)
    t_sizesN = [512] * (C // 512)
    if C % 512:
        t_sizesN.append(C % 512)

    with TileContext(nc) as tc:
        with tc.tile_pool(name="big", bufs=1) as bigp, \
             tc.tile_pool(name="w1p", bufs=3) as w1p, \
             tc.tile_pool(name="w2p", bufs=3) as w2p, \
             tc.tile_pool(name="h", bufs=3) as hp, \
             tc.tile_pool(name="fin", bufs=8) as finp, \
             tc.tile_pool(name="ps1", bufs=4, space="PSUM") as ps1p, \
             tc.tile_pool(name="ps2", bufs=4, space="PSUM") as ps2p:
            x_t = bigp.tile([P, DK, C], bf16)
            acc = bigp.tile([P, MT, HIDDEN], f32)
            warm = bigp.tile([P, 512], bf16)
            w1h = bigp.tile([P, FM, DK, P], bf16)
            x0p = bigp.tile([P, DK, P], bf16)

            x_r = xt_d[:].rearrange("(ko ki) t -> ki ko t", ki=P)
            w1h_r = w1h_d[:].rearrange("p (fm ko f) -> p fm ko f", fm=FM, ko=DK)

            # -- critical-path DMAs: q0's w1 host-packed as 4 dense 728ns
            #    chunks + the first 128 tokens packed; then 512B-run chunks.
            nc.sync.dma_start(out=w1h[:, 0], in_=w1h_r[:, 0])
            nc.sync.dma_start(out=x0p[:], in_=x0p_d[:].rearrange("p (ko t) -> p ko t", ko=DK))
            for fm in range(1, FM):
                nc.sync.dma_start(out=w1h[:, fm], in_=w1h_r[:, fm])
            nc.sync.dma_start(out=x_t[:, :, 128:384], in_=x_r[:, :, 128:384])
            w2q0 = w2p.tile([P, FM, HIDDEN], bf16, tag="w2q")
            w2r0 = w2t_d[0:FQ_SIZE, :].rearrange("(fo fi) d -> fi fo d", fi=P)
            for fk in range(FM):
                nc.sync.dma_start(out=w2q0[:, fk], in_=w2r0[:, fk])
            for off in range(384, C, 512):
                sz = min(512, C - off)
                nc.sync.dma_start(out=x_t[:, :, off:off + sz],
                                  in_=x_r[:, :, off:off + sz])
            # tokens 0:128 land in x_t too (q>=1 reads them from x_t)
            nc.sync.dma_start(out=x_t[:, :, 0:128], in_=x_r[:, :, 0:128])

            # -- PE warm-up: matmuls on a zeroed tile keep the PE busy (and
            #    ramp its p-state) while the first loads land.
            nc.vector.memset(warm[:], 0.0)
            wps = ps1p.tile([P, 512], f32, tag="ph", name="wps")
            for i in range(nwarm):
                nc.tensor.matmul(wps[:], warm[:, 0:P], warm[:],
                                 start=(i == 0), stop=(i == nwarm - 1))

            o_r = o_d[:].rearrange("(mo p) d -> p mo d", p=P)

            def do_mm1(q, w1q, t_off, t_size):
                h_t = hp.tile([P, FM, 512], bf16, tag="h", name="h_t")[:, :, :t_size]
                for fm in range(FM):
                    ph = ps1p.tile([P, 512], f32, tag="ph", name="ph")[:, :t_size]
                    for k in range(DK):
                        w1ap = (w1h[:, fm, k] if q == 0
                                else w1q[:, k, fm * P:(fm + 1) * P])
                        xap = (x0p[:, k] if q == 0 and t_off == 0
                               else x_t[:, k, t_off:t_off + t_size])
                        nc.tensor.matmul(ph[:], w1ap, xap,
                                         start=(k == 0), stop=(k == DK - 1))
                    nc.scalar.activation(h_t[:, fm], ph[:], act)
                return h_t

            def do_mm2(q, w2q, t_off, t_size, h_t):
                for tm in range(t_size // P):
                    mt = t_off // P + tm
                    last_mt = (q == FQ - 1) and (mt == MT - 1)
                    for dn in range(DN):
                        dcol = slice(dn * 512, (dn + 1) * 512)
                        if last_mt and dn == DN - 1:
                            # final chain: two 256-wide psum groups so the
                            # first half's add+store overlaps the second
                            # half's matmuls
                            for hh, dq in ((0, nc.scalar), (1, nc.sync)):
                                cl = slice(dn * 512 + hh * 256,
                                           dn * 512 + (hh + 1) * 256)
                                po = ps2p.tile([P, 512], f32, tag="po",
                                               name="po")[:, 0:256]
                                for fk in range(FM):
                                    nc.tensor.matmul(
                                        po[:], h_t[:, fk, tm * P:(tm + 1) * P],
                                        w2q[:, fk, cl],
                                        start=(fk == 0), stop=(fk == FM - 1))
                                fin = finp.tile([P, 512], bf16, tag="fin",
                                                name="fin")[:, 0:256]
                                nc.vector.tensor_add(fin[:], acc[:, mt, cl],
                                                     po[:])
                                dq.dma_start(out=o_r[:, mt, cl], in_=fin[:])
                            continue
                        po = ps2p.tile([P, 512], f32, tag="po", name="po")
                        for fk in range(FM):
                            nc.tensor.matmul(po[:],
                                             h_t[:, fk, tm * P:(tm + 1) * P],
                                             w2q[:, fk, dcol],
                                             start=(fk == 0), stop=(fk == FM - 1))
                        if q == 0:
                            nc.vector.tensor_copy(acc[:, mt, dcol], po[:])
                        elif q == FQ - 1:
                            fin = finp.tile([P, 512], bf16, tag="fin",
                                            name="fin")
                            nc.vector.tensor_add(fin[:], acc[:, mt, dcol],
                                                 po[:])
                            dq = nc.scalar if dn == 0 else nc.sync
                            dq.dma_start(out=o_r[:, mt, dcol], in_=fin[:])
                        else:
                            nc.vector.tensor_add(acc[:, mt, dcol],
                                                 acc[:, mt, dcol], po[:])

            # flat unit list; mm1 pipelined one tile ahead of mm2 so the PE
            # sequencer always has dependency-satisfied work across tile and
            # q boundaries.
            units = []
            for q in range(FQ):
                t_sizes = t_sizes0 if q == 0 else t_sizesN
                t_offs = np.cumsum([0] + t_sizes)[:-1].tolist()
                for t_off, t_size in zip(t_offs, t_sizes):
                    units.append((q, t_off, t_size))

            w_tiles = {0: (None, w2q0)}
            pending = None
            cur_q = -1
            for q, t_off, t_size in units:
                if q != cur_q:
                    cur_q = q
                    if q > 0:
                        w1q = w1p.tile([P, DK, FQ_SIZE], bf16, tag="w1q")
                        w1r = w1t_d[:, q * FQ_SIZE:(q + 1) * FQ_SIZE].rearrange(
                            "(ko ki) f -> ki ko f", ki=P)
                        nc.sync.dma_start(out=w1q[:], in_=w1r)
                        w2q = w2p.tile([P, FM, HIDDEN], bf16, tag="w2q")
                        w2r = w2t_d[q * FQ_SIZE:(q + 1) * FQ_SIZE, :].rearrange(
                            "(fo fi) d -> fi fo d", fi=P)
                        for fk in range(FM):
                            nc.sync.dma_start(out=w2q[:, fk], in_=w2r[:, fk])
                        w_tiles[q] = (w1q, w2q)
                h_t = do_mm1(q, w_tiles[q][0], t_off, t_size)
                if pending is not None:
                    pq, pt_off, pt_size, ph_t = pending
                    do_mm2(pq, w_tiles[pq][1], pt_off, pt_size, ph_t)
                pending = (q, t_off, t_size, h_t)
            pq, pt_off, pt_size, ph_t = pending
            do_mm2(pq, w_tiles[pq][1], pt_off, pt_size, ph_t)
    nc.compile()
    return nc


_B_CACHE = {}
LAST_HW_NS = None


def _run_spmd(nc, in_maps, retries=2):
    """run_bass_kernel_spmd with retry: device crashes on this axon path are
    occasionally transient (NRT_EXEC_UNIT_UNRECOVERABLE recovers on a fresh
    attempt)."""
    last = None
    for attempt in range(retries + 1):
        try:
            return run_bass_kernel_spmd(nc, in_maps, list(range(NCORES)))
        except Exception as e:  # noqa: BLE001
            last = e
            import time as _time
            _time.sleep(2.0 * (attempt + 1))
    raise last


def _expert_ffn_nc(C):
    if C not in _B_CACHE:
        _B_CACHE[C] = _build_expert_ffn(C)
    return _B_CACHE[C]


def kernel(x, router_w, expert_w1, expert_w2):
    xf = np.ascontiguousarray(x.reshape(T, HIDDEN), dtype=np.float32)

    # ---- host dispatch: router (replicated per the hint) + top-2 softmax ----
    scores = xf @ np.ascontiguousarray(router_w.astype(np.float32)).T  # [T, E]
    top1 = np.argmax(scores, axis=1)
    rng = np.arange(T)
    s1 = scores[rng, top1]
    scores2 = scores.copy()
    scores2[rng, top1] = -np.inf
    top2 = np.argmax(scores2, axis=1)
    s2 = scores2[rng, top2]
    g1 = 1.0 / (1.0 + np.exp(-(s1 - s2)))     # softmax over the two slots
    w_all = np.zeros((T, NUM_EXPERTS), dtype=np.float32)
    w_all[rng, top1] = g1
    w_all[rng, top2] = 1.0 - g1

    idx = [np.nonzero((top1 == e) | (top2 == e))[0] for e in range(NUM_EXPERTS)]
    cmax = max(len(i) for i in idx)
    C = min(((cmax + P - 1) // P) * P, MAXC)
    n_chunks = (cmax + C - 1) // C

    xT16 = _to_bf16(xf).T                      # [D, T] bf16 view (strided ok)
    w1t16 = [np.ascontiguousarray(_to_bf16(expert_w1[e]).T) for e in range(NUM_EXPERTS)]
    w2t16 = [np.ascontiguousarray(_to_bf16(expert_w2[e]).T) for e in range(NUM_EXPERTS)]
    # w1h: w1t[:, 0:512] packed [ki][(fm ko f)] (dense rows for a fast start)
    w1h16 = [np.ascontiguousarray(
        w[:, :FQ_SIZE].reshape(DK, P, FM, P).transpose(1, 2, 0, 3)
        .reshape(P, FM * DK * P)) for w in w1t16]
    nc_b = _expert_ffn_nc(C)

    out = np.zeros((T, HIDDEN), dtype=np.float32)
    for r in range(n_chunks):
        in_b = []
        for e in range(NUM_EXPERTS):
            ids = idx[e][r * C:(r + 1) * C]
            n = len(ids)
            xsel = np.zeros((HIDDEN, C), dtype=ml_dtypes.bfloat16)
            xsel[:, :n] = xT16[:, ids]
            x0p = np.ascontiguousarray(
                xsel[:, :P].reshape(DK, P, P).transpose(1, 0, 2).reshape(P, DK * P))
            in_b.append({"xt": xsel, "w1h": w1h16[e], "x0p": x0p,
                         "w1t": w1t16[e], "w2t": w2t16[e]})
        # ---- expert-parallel FFN on device (single launch per chunk) ----
        res_b = _run_spmd(nc_b, in_b)
        # ---- host combine: gated scatter-add (ids unique per expert) ----
        for e in range(NUM_EXPERTS):
            ids = idx[e][r * C:(r + 1) * C]
            n = len(ids)
            o16 = np.asarray(res_b.results[e]["o"][:n], dtype=np.float32)
            out[ids] += o16 * w_all[ids, e][:, None]

    # cost-model exec-time estimate (NTFF profiling unavailable on this path)
    global LAST_HW_NS
    try:
        if ("t", C) not in _B_CACHE:
            from concourse.timeline_sim import TimelineSim
            _B_CACHE[("t", C)] = TimelineSim(nc_b).simulate() * n_chunks
        LAST_HW_NS = int(_B_CACHE[("t", C)])
    except Exception:  # noqa: BLE001
        pass
    return out.reshape(BATCH, SEQ, HIDDEN)


# revision 27
# speedup vs baseline: 1.0043x; 1.0004x over previous
"""MoE (top-2 of 8 experts, gelu MLP) on 8 TRN2 NeuronCores.

Strategy (expert-parallel, per the sharding hint):
  Host dispatch: router scores/top-2/softmax on host (the router is 0.05% of
    the FLOPs; "replicate router" per the hint — the host mediates the
    all-to-all anyway). Each expert's selected token columns are gathered
    into a capacity-padded bf16 batch.
  Device (expert-parallel, one launch): each core runs one expert's FFN
    out = gelu(xsel @ w1[e].T) @ w2[e].T over its gathered tokens in bf16
    (fp32 PSUM accumulation), with:
      - x resident in SBUF across all 8 FFN column blocks (no re-DMA),
      - PE warm-up matmuls covering the initial DMA wait + p-state ramp,
      - q0's w1 and the first tokens host-packed dense (728ns DMAs) and the
        first two token tiles' mm1 hoisted ahead of mm2, so real matmuls
        start ~4.3us in and never starve on the w2 load,
      - output written during the last FFN block (acc add then DMA per
        512-wide half, alternating the two HWDGE queues; the final chain
        split into 256-wide halves) to minimize the tail.
  Host combine: scatter-add with the routing weight applied during the
  combine (each token appears in exactly two experts' batches).

kernel(**inputs) -> np.ndarray  takes FULL inputs, returns FULL output.
"""

import numpy as np
import ml_dtypes

import concourse.bass as bass
import concourse.mybir as mybir
from concourse import bacc
from concourse.tile import TileContext
from concourse.bass_utils import run_bass_kernel_spmd

HIDDEN = 1024
NUM_EXPERTS = 8
TOP_K = 2
FFN = 4096
BATCH, SEQ = 4, 2048
T = BATCH * SEQ          # 8192 tokens
NCORES = 8
P = 128
DK = HIDDEN // P         # 8 contraction tiles over hidden
FQ = 8                   # FFN column blocks
FQ_SIZE = FFN // FQ      # 512
FM = FQ_SIZE // P        # 4 f-subtiles per block
DN = HIDDEN // 512       # 2 output column halves
MAXC = 2560              # SBUF limit for resident x + fp32 accumulator
NWARM = 10               # PE warm-up matmuls (512 rows each)

f32 = mybir.dt.float32
bf16 = mybir.dt.bfloat16


def _to_bf16(a: np.ndarray) -> np.ndarray:
    """fp32 -> bf16 with round-to-nearest-even (fast uint path)."""
    u = np.ascontiguousarray(a, dtype=np.float32).view(np.uint32)
    r = ((u + np.uint32(0x7FFF) + ((u >> np.uint32(16)) & np.uint32(1)))
         >> np.uint32(16)).astype(np.uint16)
    return r.view(ml_dtypes.bfloat16)


def _build_expert_ffn(C: int, nwarm: int = NWARM):
    """Per core: one expert's FFN over C gathered tokens (bf16 matmuls).

    inputs: xt  [HIDDEN, C] bf16    gathered+transposed tokens
            w1h [P, FM*DK*P] bf16   w1t[:, 0:512] host-packed [ki][(fm ko f)]
            x0p [P, DK*P] bf16      xt[:, 0:128] host-packed [ki][(ko t)]
            w1t [HIDDEN, FFN] bf16  expert w1 transposed
            w2t [FFN, HIDDEN] bf16  expert w2 transposed
    output: o   [C, HIDDEN] bf16    gelu(x @ w1.T) @ w2.T   (ungated)
    """
    assert C % P == 0 and C >= 1024
    act = mybir.ActivationFunctionType.Gelu
    nc = bacc.Bacc(None)
    xt_d = nc.declare_dram_parameter("xt", [HIDDEN, C], bf16, isOutput=False)
    w1h_d = nc.declare_dram_parameter("w1h", [P, FM * DK * P], bf16, isOutput=False)
    x0p_d = nc.declare_dram_parameter("x0p", [P, DK * P], bf16, isOutput=False)
    w1t_d = nc.declare_dram_parameter("w1t", [HIDDEN, FFN], bf16, isOutput=False)
    w2t_d = nc.declare_dram_parameter("w2t", [FFN, HIDDEN], bf16, isOutput=False)
    o_d = nc.declare_dram_parameter("o", [C, HIDDEN], bf16, isOutput=True)

    MT = C // P
    # token tiles: q0 starts fine-grained (128 host-packed + two 256s) so the
    # first matmuls only wait for small DMAs; later q use full 512 tiles.
    t_sizes0 = [128, 256, 256] + [512] * ((C - 640) // 512)
    if (C - 640) % 512:
        t_sizes0.append((C - 640) % 512)
    t_sizesN = [512] * (C // 512)
    if C % 512:
        t_sizesN.append(C % 512)

    with TileContext(nc) as tc:
        with tc.tile_pool(name="big", bufs=1) as bigp, \
             tc.tile_pool(name="w1p", bufs=3) as w1p, \
             tc.tile_pool(name="w2p", bufs=3) as w2p, \
             tc.tile_pool(name="h", bufs=3) as hp, \
             tc.tile_pool(name="fin", bufs=8) as finp, \
             tc.tile_pool(name="ps1", bufs=4, space="PSUM") as ps1p, \
             tc.tile_pool(name="ps2", bufs=4, space="PSUM") as ps2p:
            x_t = bigp.tile([P, DK, C], bf16)
            acc = bigp.tile([P, MT, HIDDEN], f32)
            warm = bigp.tile([P, 512], bf16)
            w1h = bigp.tile([P, FM, DK, P], bf16)
            x0p = bigp.tile([P, DK, P], bf16)

            x_r = xt_d[:].rearrange("(ko ki) t -> ki ko t", ki=P)
            w1h_r = w1h_d[:].rearrange("p (fm ko f) -> p fm ko f", fm=FM, ko=DK)

            # -- critical-path DMAs: q0's w1 host-packed as 4 dense 728ns
            #    chunks + the first 128 tokens packed; then 512B-run chunks.
            nc.sync.dma_start(out=w1h[:, 0], in_=w1h_r[:, 0])
            nc.sync.dma_start(out=x0p[:], in_=x0p_d[:].rearrange("p (ko t) -> p ko t", ko=DK))
            for fm in range(1, FM):
                nc.sync.dma_start(out=w1h[:, fm], in_=w1h_r[:, fm])
            nc.sync.dma_start(out=x_t[:, :, 128:384], in_=x_r[:, :, 128:384])
            w2q0 = w2p.tile([P, FM, HIDDEN], bf16, tag="w2q")
            w2r0 = w2t_d[0:FQ_SIZE, :].rearrange("(fo fi) d -> fi fo d", fi=P)
            for fk in range(FM):
                nc.sync.dma_start(out=w2q0[:, fk], in_=w2r0[:, fk])
            for off in range(384, C, 512):
                sz = min(512, C - off)
                nc.sync.dma_start(out=x_t[:, :, off:off + sz],
                                  in_=x_r[:, :, off:off + sz])
            # tokens 0:128 land in x_t too (q>=1 reads them from x_t)
            nc.sync.dma_start(out=x_t[:, :, 0:128], in_=x_r[:, :, 0:128])

            # -- PE warm-up: matmuls on a zeroed tile keep the PE busy (and
            #    ramp its p-state) while the first loads land.
            nc.vector.memset(warm[:], 0.0)
            wps = ps1p.tile([P, 512], f32, tag="ph", name="wps")
            for i in range(nwarm):
                nc.tensor.matmul(wps[:], warm[:, 0:P], warm[:],
                                 start=(i == 0), stop=(i == nwarm - 1))

            o_r = o_d[:].rearrange("(mo p) d -> p mo d", p=P)

            def do_mm1(q, w1q, t_off, t_size):
                h_t = hp.tile([P, FM, 512], bf16, tag="h", name="h_t")[:, :, :t_size]
                for fm in range(FM):
                    ph = ps1p.tile([P, 512], f32, tag="ph", name="ph")[:, :t_size]
                    for k in range(DK):
                        w1ap = (w1h[:, fm, k] if q == 0
                                else w1q[:, k, fm * P:(fm + 1) * P])
                        xap = (x0p[:, k] if q == 0 and t_off == 0
                               else x_t[:, k, t_off:t_off + t_size])
                        nc.tensor.matmul(ph[:], w1ap, xap,
                                         start=(k == 0), stop=(k == DK - 1))
                    nc.scalar.activation(h_t[:, fm], ph[:], act)
                return h_t

            def do_mm2(q, w2q, t_off, t_size, h_t):
                for tm in range(t_size // P):
                    mt = t_off // P + tm
                    last_mt = (q == FQ - 1) and (mt == MT - 1)
                    for dn in range(DN):
                        dcol = slice(dn * 512, (dn + 1) * 512)
                        if last_mt and dn == DN - 1:
                            # final chain: two 256-wide psum groups so the
                            # first half's add+store overlaps the second
                            # half's matmuls
                            for hh, dq in ((0, nc.scalar), (1, nc.gpsimd)):
                                cl = slice(dn * 512 + hh * 256,
                                           dn * 512 + (hh + 1) * 256)
                                po = ps2p.tile([P, 512], f32, tag="po",
                                               name="po")[:, 0:256]
                                for fk in range(FM):
                                    nc.tensor.matmul(
                                        po[:], h_t[:, fk, tm * P:(tm + 1) * P],
                                        w2q[:, fk, cl],
                                        start=(fk == 0), stop=(fk == FM - 1))
                                fin = finp.tile([P, 512], bf16, tag="fin",
                                                name="fin")[:, 0:256]
                                nc.vector.tensor_add(fin[:], acc[:, mt, cl],
                                                     po[:])
                                dq.dma_start(out=o_r[:, mt, cl], in_=fin[:])
                            continue
                        po = ps2p.tile([P, 512], f32, tag="po", name="po")
                        for fk in range(FM):
                            nc.tensor.matmul(po[:],
                                             h_t[:, fk, tm * P:(tm + 1) * P],
                                             w2q[:, fk, dcol],
                                             start=(fk == 0), stop=(fk == FM - 1))
                        if q == 0:
                            nc.vector.tensor_copy(acc[:, mt, dcol], po[:])
                        elif q == FQ - 1:
                            fin = finp.tile([P, 512], bf16, tag="fin",
                                            name="fin")
                            nc.vector.tensor_add(fin[:], acc[:, mt, dcol],
                                                 po[:])
                            dq = nc.scalar if dn == 0 else nc.sync
                            dq.dma_start(out=o_r[:, mt, dcol], in_=fin[:])
                        else:
                            nc.vector.tensor_add(acc[:, mt, dcol],
                                                 acc[:, mt, dcol], po[:])

            # flat unit list; mm1 pipelined one tile ahead of mm2 so the PE
            # sequencer always has dependency-satisfied work across tile and
            # q boundaries.
            units = []
            for q in range(FQ):
                t_sizes = t_sizes0 if q == 0 else t_sizesN
                t_offs = np.cumsum([0] + t_sizes)[:-1].tolist()
                for t_off, t_size in zip(t_offs, t_sizes):
                    units.append((q, t_off, t_size))

            w_tiles = {0: (None, w2q0)}
            pending = None
            cur_q = -1
            for q, t_off, t_size in units:
                if q != cur_q:
                    cur_q = q
                    if q > 0:
                        w1q = w1p.tile([P, DK, FQ_SIZE], bf16, tag="w1q")
                        w1r = w1t_d[:, q * FQ_SIZE:(q + 1) * FQ_SIZE].rearrange(
                            "(ko ki) f -> ki ko f", ki=P)
                        nc.sync.dma_start(out=w1q[:], in_=w1r)
                        w2q = w2p.tile([P, FM, HIDDEN], bf16, tag="w2q")
                        w2r = w2t_d[q * FQ_SIZE:(q + 1) * FQ_SIZE, :].rearrange(
                            "(fo fi) d -> fi fo d", fi=P)
                        for fk in range(FM):
                            nc.sync.dma_start(out=w2q[:, fk], in_=w2r[:, fk])
                        w_tiles[q] = (w1q, w2q)
                h_t = do_mm1(q, w_tiles[q][0], t_off, t_size)
                if pending is not None:
                    pq, pt_off, pt_size, ph_t = pending
                    do_mm2(pq, w_tiles[pq][1], pt_off, pt_size, ph_t)
                pending = (q, t_off, t_size, h_t)
            pq, pt_off, pt_size, ph_t = pending
            do_mm2(pq, w_tiles[pq][1], pt_off, pt_size, ph_t)
    nc.compile()
    return nc


_B_CACHE = {}
LAST_HW_NS = None


def _run_spmd(nc, in_maps, retries=2):
    """run_bass_kernel_spmd with retry: device crashes on this axon path are
    occasionally transient (NRT_EXEC_UNIT_UNRECOVERABLE recovers on a fresh
    attempt)."""
    last = None
    for attempt in range(retries + 1):
        try:
            return run_bass_kernel_spmd(nc, in_maps, list(range(NCORES)))
        except Exception as e:  # noqa: BLE001
            last = e
            import time as _time
            _time.sleep(2.0 * (attempt + 1))
    raise last


def _expert_ffn_nc(C):
    if C not in _B_CACHE:
        _B_CACHE[C] = _build_expert_ffn(C)
    return _B_CACHE[C]


def kernel(x, router_w, expert_w1, expert_w2):
    xf = np.ascontiguousarray(x.reshape(T, HIDDEN), dtype=np.float32)

    # ---- host dispatch: router (replicated per the hint) + top-2 softmax ----
    scores = xf @ np.ascontiguousarray(router_w.astype(np.float32)).T  # [T, E]
    top1 = np.argmax(scores, axis=1)
    rng = np.arange(T)
    s1 = scores[rng, top1]
    scores2 = scores.copy()
    scores2[rng, top1] = -np.inf
    top2 = np.argmax(scores2, axis=1)
    s2 = scores2[rng, top2]
    g1 = 1.0 / (1.0 + np.exp(-(s1 - s2)))     # softmax over the two slots
    w_all = np.zeros((T, NUM_EXPERTS), dtype=np.float32)
    w_all[rng, top1] = g1
    w_all[rng, top2] = 1.0 - g1

    idx = [np.nonzero((top1 == e) | (top2 == e))[0] for e in range(NUM_EXPERTS)]
    cmax = max(len(i) for i in idx)
    C = min(((cmax + P - 1) // P) * P, MAXC)
    n_chunks = (cmax + C - 1) // C

    xT16 = _to_bf16(xf).T                      # [D, T] bf16 view (strided ok)
    w1t16 = [np.ascontiguousarray(_to_bf16(expert_w1[e]).T) for e in range(NUM_EXPERTS)]
    w2t16 = [np.ascontiguousarray(_to_bf16(expert_w2[e]).T) for e in range(NUM_EXPERTS)]
    # w1h: w1t[:, 0:512] packed [ki][(fm ko f)] (dense rows for a fast start)
    w1h16 = [np.ascontiguousarray(
        w[:, :FQ_SIZE].reshape(DK, P, FM, P).transpose(1, 2, 0, 3)
        .reshape(P, FM * DK * P)) for w in w1t16]
    nc_b = _expert_ffn_nc(C)

    out = np.zeros((T, HIDDEN), dtype=np.float32)
    for r in range(n_chunks):
        in_b = []
        for e in range(NUM_EXPERTS):
            ids = idx[e][r * C:(r + 1) * C]
            n = len(ids)
            xsel = np.zeros((HIDDEN, C), dtype=ml_dtypes.bfloat16)
            xsel[:, :n] = xT16[:, ids]
            x0p = np.ascontiguousarray(
                xsel[:, :P].reshape(DK, P, P).transpose(1, 0, 2).reshape(P, DK * P))
            in_b.append({"xt": xsel, "w1h": w1h16[e], "x0p": x0p,
                         "w1t": w1t16[e], "w2t": w2t16[e]})
        # ---- expert-parallel FFN on device (single launch per chunk) ----
        res_b = _run_spmd(nc_b, in_b)
        # ---- host combine: gated scatter-add (ids unique per expert) ----
        for e in range(NUM_EXPERTS):
            ids = idx[e][r * C:(r + 1) * C]
            n = len(ids)
            o16 = np.asarray(res_b.results[e]["o"][:n], dtype=np.float32)
            out[ids] += o16 * w_all[ids, e][:, None]

    # cost-model exec-time estimate (NTFF profiling unavailable on this path)
    global LAST_HW_NS
    try:
        if ("t", C) not in _B_CACHE:
            from concourse.timeline_sim import TimelineSim
            _B_CACHE[("t", C)] = TimelineSim(nc_b).simulate() * n_chunks
        LAST_HW_NS = int(_B_CACHE[("t", C)])
    except Exception:  # noqa: BLE001
        pass
    return out.reshape(BATCH, SEQ, HIDDEN)
